# revision 14
# baseline (speedup 1.0000x reference)
"""Trainium2 Bass kernel for nn_ContextEncoder.

Pipeline (per sample b): feature transform tanh(X @ W_t.T + b_t), a
"bidirectional" LSTM where both directions run forward (matching the
reference), attention pooling against the last hidden state, and a
context norm over the flattened (d, 2h) vector.

Sharding: data-parallel over b (16 samples -> 2 per core on 8 cores).
Each core runs 128 independent sequences (2 b x 64 d) of length T=128.

Per-core layout choices:
  - xs (LSTM inputs) stored [ts=64 (+ ones row), (t, b, d)]: gates/hidden
    live on partitions, batch on the free dim, so the recurrence needs no
    transposes.
  - gate preacts accumulate in PSUM: psum[gate_chunk, batch] =
    Wih.T|bias @ [xs;1] (K=65) + Whh.T @ h (K=128); all matmul operands
    bf16, fp32 accumulation.
  - Gate order per bank is (i, f, o, g) so one sigmoid covers [0:384].
  - h tiles are DMA-xbar-transposed each step into HT [j, t, 2h] for the
    attention-pooling tail.
"""

import sys

for _p in ("/opt/trn_rl_repo", "/root/.axon_site/_ro/trn_rl_repo"):
    if _p not in sys.path:
        sys.path.append(_p)

import numpy as np
import ml_dtypes

import concourse.bass as bass
import concourse.bacc as bacc
import concourse.tile as tile
from concourse import mybir
from concourse.bass_utils import run_bass_kernel_spmd

BF16NP = ml_dtypes.bfloat16
F32 = mybir.dt.float32
F32R = mybir.dt.float32r
BF16 = mybir.dt.bfloat16
AF = mybir.ActivationFunctionType
ALU = mybir.AluOpType

B, T, D, NF = 16, 128, 64, 32
TS, H = 64, 128
NCORES = 8
BLOC = B // NCORES          # 2 samples per core
J = BLOC * D                # 128 sequences per core
R = J * T                   # 16384 (t, b, d) columns
G4 = 4 * H                  # 512 gates per direction
PERM = (0, 1, 3, 2)         # torch gate order (i,f,g,o) -> (i,f,o,g)
NORM_N = D * 2 * H          # 16384 context-norm elements per sample


def emit(tc, ins, outs):
    nc = tc.nc
    # h goes SBUF->DRAM->SBUF for the transpose: the DRAM->SBUF xbar path is
    # the hardware-validated one (direct SBUF->SBUF transposes read stale data)
    HSC = [nc.dram_tensor(f"hscr{d}", [T, H, J], BF16).ap() for d in range(2)]
    XT, WTT, BT = ins["XT"], ins["WTT"], ins["BT"]
    if XT.dtype == F32:
        XT = XT.bitcast(F32R)
    if WTT.dtype == F32:
        WTT = WTT.bitcast(F32R)
    WIH, WHH, ONES = ins["WIH"], ins["WHH"], ins["ONES"]
    DW, DB = ins["DW"], ins["DB"]
    OUT = outs["OUT"]

    with (
        tc.tile_pool(name="consts", bufs=1) as consts,
        tc.tile_pool(name="cpool", bufs=2) as cpool,
        tc.tile_pool(name="sgpool", bufs=2) as sgpool,
        tc.tile_pool(name="small", bufs=2) as small,
        tc.tile_pool(name="gates", bufs=2, space="PSUM") as gates,
    ):
        # ---- constants / weights ----
        wtt = consts.tile([NF, TS], F32R)
        nc.sync.dma_start(wtt, WTT)
        bt = consts.tile([TS, 1], F32)
        nc.sync.dma_start(bt, BT)
        wih = consts.tile([TS + 1, 2, G4], BF16)
        nc.sync.dma_start(wih, WIH)
        whh = consts.tile([H, 2, G4], BF16)
        nc.sync.dma_start(whh, WHH)
        # HT: attention layout [j, t, 2h] filled by per-step DMA transposes
        ht = consts.tile([J, T, 2 * H], BF16)

        with (
            tc.tile_pool(name="xtp", bufs=4) as xtp,
            tc.tile_pool(name="tfp", bufs=2, space="PSUM") as tfp,
            tc.tile_pool(name="xs2p", bufs=1) as xs2p,
            tc.tile_pool(name="hslabs", bufs=1) as hslabs,
        ):
            # h history slabs: transpose sources are never recycled
            hsl = [hslabs.tile([H, R], BF16, tag=f"hs{d}", name=f"hs{d}")
                   for d in range(2)]
            h_prev = [None, None]
            c_prev = [None, None]
            for d in range(2):
                h0 = hslabs.tile([H, J], BF16, tag=f"hz{d}")
                nc.vector.memset(h0, 0.0)
                c0 = cpool.tile([H, J], BF16, tag=f"c{d}")
                nc.vector.memset(c0, 0.0)
                h_prev[d] = h0
                c_prev[d] = c0
            # ---- feature transform: xs2[0:64, (t,b,d)] = tanh(Wt @ X.T + bt)
            xs2 = xs2p.tile([TS + 1, R], BF16)
            nc.sync.dma_start(xs2[TS : TS + 1, :], ONES)
            for cc in range(R // 512):
                xt = xtp.tile([NF, 512], F32R, tag="xt")
                nc.sync.dma_start(xt, XT[:, cc * 512 : (cc + 1) * 512])
                pz = tfp.tile([TS, 512], F32, tag="pz")
                nc.tensor.matmul(pz, lhsT=wtt, rhs=xt, start=True, stop=True)
                nc.scalar.activation(
                    out=xs2[0:TS, cc * 512 : (cc + 1) * 512],
                    in_=pz, func=AF.Tanh, bias=bt, scale=1.0,
                )

            if "DBG_XS" in outs:
                nc.sync.dma_start(outs["DBG_XS"], xs2[0:TS, 0:512])
            # ---- recurrence ----
            for t in range(T):
                psg = [None, None]
                for d in range(2):
                    pg = gates.tile([H, G4], F32, tag=f"g{d}")
                    psg[d] = pg
                    rhs_x = xs2[:, t * J : (t + 1) * J]
                    for c in range(4):
                        nc.tensor.matmul(
                            pg[:, c * H : (c + 1) * H],
                            lhsT=wih[:, d, c * H : (c + 1) * H],
                            rhs=rhs_x, start=(c == 0), stop=False,
                        )
                for d in range(2):
                    for c in range(4):
                        nc.tensor.matmul(
                            psg[d][:, c * H : (c + 1) * H],
                            lhsT=whh[:, d, c * H : (c + 1) * H],
                            rhs=h_prev[d], start=False, stop=(c == 3),
                        )
                for d in range(2):
                    pg = psg[d]
                    sg = sgpool.tile([H, 3 * H], BF16, tag=f"sg{d}")
                    nc.scalar.activation(out=sg, in_=pg[:, 0 : 3 * H], func=AF.Sigmoid)
                    tg = small.tile([H, J], BF16, tag=f"tg{d}")
                    nc.scalar.activation(out=tg, in_=pg[:, 3 * H : G4], func=AF.Tanh)
                    c2 = small.tile([H, J], BF16, tag=f"c2{d}")
                    nc.vector.tensor_mul(c2, sg[:, H : 2 * H], c_prev[d])
                    u = small.tile([H, J], BF16, tag=f"u{d}")
                    nc.gpsimd.tensor_mul(u, sg[:, 0:H], tg)
                    cn = cpool.tile([H, J], BF16, tag=f"c{d}")
                    nc.vector.tensor_add(cn, c2, u)
                    tch = small.tile([H, J], BF16, tag=f"tc{d}")
                    nc.scalar.activation(out=tch, in_=cn, func=AF.Tanh)
                    hn = hsl[d][:, t * J : (t + 1) * J]
                    nc.vector.tensor_mul(hn, sg[:, 2 * H : 3 * H], tch)
                    if "DBG_H0" in outs and t == 0 and d == 0:
                        nc.sync.dma_start(outs["DBG_H0"], hn)
                    nc.sync.dma_start(HSC[d][t], hn)
                    nc.sync.dma_start_transpose(
                        ht[:, t, d * H : (d + 1) * H], HSC[d][t]
                    )
                    if "DBG_HT" in outs and t == 1 and d == 1:
                        nc.sync.dma_start(outs["DBG_HT"], ht[:, 0, :])
                    h_prev[d] = hn
                    c_prev[d] = cn

        # ---- tail: attention pooling + context norm ----
        with (
            tc.tile_pool(name="tailp", bufs=1) as tailp,
            tc.tile_pool(name="tailps", bufs=1, space="PSUM") as tailps,
        ):
            htj = ht[:, T - 1, :]  # [J, 2H] last hidden state
            htj_b = bass.AP(
                tensor=htj.tensor, offset=htj.offset,
                ap=[list(htj.ap[0]), [0, T], list(htj.ap[-1])],
            )
            prod = tailp.tile([J, T, 2 * H], BF16)
            nc.vector.tensor_mul(prod, ht, htj_b)
            logits = tailp.tile([J, T], F32)
            nc.vector.tensor_reduce(logits, prod, axis=mybir.AxisListType.X, op=ALU.add)
            mx = tailp.tile([J, 1], F32)
            nc.vector.tensor_reduce(mx, logits, axis=mybir.AxisListType.X, op=ALU.max)
            mxn = tailp.tile([J, 1], F32)
            nc.vector.tensor_scalar_mul(mxn, mx, -1.0)
            ew = tailp.tile([J, T], F32)
            dsum = tailp.tile([J, 1], F32)
            nc.scalar.activation(out=ew, in_=logits, func=AF.Exp, bias=mxn,
                                 scale=1.0, accum_out=dsum)
            rd = tailp.tile([J, 1], F32)
            nc.vector.reciprocal(rd, dsum)
            nc.vector.tensor_scalar_mul(ew, ew, rd)  # softmax weights in place
            ew_b = bass.AP(
                tensor=ew.tensor, offset=ew.offset,
                ap=[list(ew.ap[0]), list(ew.ap[-1]), [0, 2 * H]],
            )
            prod2 = tailp.tile([J, T, 2 * H], BF16, tag="prod")  # reuse slab
            nc.vector.tensor_mul(prod2, ht, ew_b)
            # reduce over t keeping p: view [j, p, t]
            p2v = bass.AP(
                tensor=prod2.tensor, offset=prod2.offset,
                ap=[list(prod2.ap[0]), [1, 2 * H], [2 * H, T]],
            )
            pooled = tailp.tile([J, 2 * H], F32)
            nc.vector.tensor_reduce(pooled, p2v, axis=mybir.AxisListType.X, op=ALU.add)
            if "DBG_LOG" in outs:
                nc.sync.dma_start(outs["DBG_LOG"], logits)
                nc.sync.dma_start(outs["DBG_PO"], pooled)

            # context norm across each sample's (d, 2h) block
            pooled2 = tailp.tile([J, 2 * H], F32)
            nc.scalar.activation(out=pooled2, in_=pooled, func=AF.Square)
            sel = tailp.tile([J, BLOC], F32)
            nc.sync.dma_start(sel, ins["SEL"])
            pstat = tailps.tile([BLOC, 2 * G4], F32, tag="stats")
            nc.tensor.matmul(pstat[:, 0 : 2 * H], lhsT=sel, rhs=pooled,
                             start=True, stop=False)
            nc.tensor.matmul(pstat[:, 2 * H : 4 * H], lhsT=sel, rhs=pooled2,
                             start=False, stop=True)
            s1 = tailp.tile([BLOC, 1], F32)
            nc.vector.tensor_reduce(s1, pstat[:, 0 : 2 * H],
                                    axis=mybir.AxisListType.X, op=ALU.add)
            s2 = tailp.tile([BLOC, 1], F32)
            nc.vector.tensor_reduce(s2, pstat[:, 2 * H : 4 * H],
                                    axis=mybir.AxisListType.X, op=ALU.add)
            stats2 = tailp.tile([BLOC, 2], F32)
            nc.scalar.mul(stats2[:, 0:1], s1, 1.0 / NORM_N)      # mean
            q = tailp.tile([BLOC, 1], F32)
            nc.vector.tensor_mul(q, s1, stats2[:, 0:1])          # sum*mean
            v = tailp.tile([BLOC, 1], F32)
            nc.vector.tensor_tensor(v, s2, q, op=ALU.subtract)
            sd = tailp.tile([BLOC, 1], F32)
            nc.scalar.activation(out=sd, in_=v, func=AF.Sqrt,
                                 scale=1.0 / (NORM_N - 1))
            nc.vector.reciprocal(stats2[:, 1:2], sd)             # rstd
            selt = tailp.tile([BLOC, J], F32)
            nc.sync.dma_start(selt, ins["SELT"])
            pmb = tailps.tile([J, 2], F32, tag="mb")
            nc.tensor.matmul(pmb, lhsT=selt, rhs=stats2, start=True, stop=True)
            mb = tailp.tile([J, 2], F32)
            nc.vector.tensor_copy(mb, pmb)
            dwt = tailp.tile([J, 2 * H], F32)
            nc.sync.dma_start(dwt[0:D, :], DW)
            nc.sync.dma_start(dwt[D:J, :], DW)
            dbt = tailp.tile([J, 2 * H], F32)
            nc.sync.dma_start(dbt[0:D, :], DB)
            nc.sync.dma_start(dbt[D:J, :], DB)
            t1 = tailp.tile([J, 2 * H], F32)
            nc.vector.tensor_scalar(t1, pooled, mb[:, 0:1], mb[:, 1:2],
                                    op0=ALU.subtract, op1=ALU.mult)
            t2 = tailp.tile([J, 2 * H], F32)
            nc.vector.tensor_mul(t2, t1, dwt)
            t3 = tailp.tile([J, 2 * H], F32)
            nc.vector.tensor_add(t3, t2, dbt)
            nc.sync.dma_start(OUT, t3)


def build_program():
    nc = bacc.Bacc("TRN2", target_bir_lowering=False, debug=False)
    ins = {
        "XT": nc.dram_tensor("XT", [NF, R], F32R, kind="ExternalInput").ap(),
        "WTT": nc.dram_tensor("WTT", [NF, TS], F32R, kind="ExternalInput").ap(),
        "BT": nc.dram_tensor("BT", [TS, 1], F32, kind="ExternalInput").ap(),
        "WIH": nc.dram_tensor("WIH", [TS + 1, 2, G4], BF16, kind="ExternalInput").ap(),
        "WHH": nc.dram_tensor("WHH", [H, 2, G4], BF16, kind="ExternalInput").ap(),
        "ONES": nc.dram_tensor("ONES", [1, R], BF16, kind="ExternalInput").ap(),
        "DW": nc.dram_tensor("DW", [D, 2 * H], F32, kind="ExternalInput").ap(),
        "SEL": nc.dram_tensor("SEL", [J, BLOC], F32, kind="ExternalInput").ap(),
        "SELT": nc.dram_tensor("SELT", [BLOC, J], F32, kind="ExternalInput").ap(),
        "DB": nc.dram_tensor("DB", [D, 2 * H], F32, kind="ExternalInput").ap(),
    }
    outs = {
        "OUT": nc.dram_tensor("OUT", [J, 2 * H], F32, kind="ExternalOutput").ap(),
    }
    with tile.TileContext(nc) as tc:
        emit(tc, ins, outs)
    nc.compile()
    return nc


def _prep_dir(Wih, Whh, bih, bhh):
    wihT = Wih.T.reshape(TS, 4, H)[:, PERM, :].reshape(TS, G4)
    biasr = (bih + bhh).reshape(4, H)[PERM, :].reshape(G4)
    wih65 = np.concatenate([wihT, biasr[None, :]], axis=0).astype(BF16NP)
    whhT = Whh.T.reshape(H, 4, H)[:, PERM, :].reshape(H, G4).astype(BF16NP)
    return wih65, whhT


def prep_inputs(X, W_t, b_t, Wih_f, Whh_f, bih_f, bhh_f,
                Wih_b, Whh_b, bih_b, bhh_b, diag_w, diag_b):
    wih_f, whh_f = _prep_dir(Wih_f, Whh_f, bih_f, bhh_f)
    wih_b, whh_b = _prep_dir(Wih_b, Whh_b, bih_b, bhh_b)
    shared = {
        "WTT": np.ascontiguousarray(W_t.T, dtype=np.float32),
        "BT": np.ascontiguousarray(b_t.reshape(TS, 1), dtype=np.float32),
        "WIH": np.ascontiguousarray(np.stack([wih_f, wih_b], axis=1)),
        "WHH": np.ascontiguousarray(np.stack([whh_f, whh_b], axis=1)),
        "ONES": np.ones((1, R), dtype=BF16NP),
        "SEL": np.kron(np.eye(BLOC, dtype=np.float32), np.ones((D, 1), np.float32)),
        "SELT": np.kron(np.eye(BLOC, dtype=np.float32), np.ones((1, D), np.float32)),
        "DW": np.ascontiguousarray(diag_w.reshape(D, 2 * H), dtype=np.float32),
        "DB": np.ascontiguousarray(diag_b.reshape(D, 2 * H), dtype=np.float32),
    }
    in_maps = []
    for i in range(NCORES):
        xt = np.ascontiguousarray(
            X[i * BLOC : (i + 1) * BLOC].transpose(3, 1, 0, 2).reshape(NF, R),
            dtype=np.float32,
        )
        m = {"XT": xt}
        m.update(shared)
        in_maps.append(m)
    return in_maps


def kernel(**inputs):
    inputs = {k: np.asarray(v, dtype=np.float32) for k, v in inputs.items()}
    in_maps = prep_inputs(**inputs)
    nc = build_program()
    res = run_bass_kernel_spmd(nc, in_maps, list(range(NCORES)))
    out = np.concatenate(
        [res.results[i]["OUT"].reshape(BLOC, D, 2 * H) for i in range(NCORES)],
        axis=0,
    )
    return np.ascontiguousarray(out, dtype=np.float32)


if __name__ == "__main__":
    nc = build_program()
    print("program built ok")


# revision 17
# speedup vs baseline: 1.8096x; 1.8096x over previous
"""Trainium2 Bass kernel for nn_ContextEncoder.

Pipeline (per sample b): feature transform tanh(X @ W_t.T + b_t), a
"bidirectional" LSTM where both directions run forward (matching the
reference), attention pooling against the last hidden state, and a
context norm over the flattened (d, 2h) vector.

Sharding: data-parallel over b (16 samples -> 2 per core on 8 cores).
Each core runs 128 independent sequences (2 b x 64 d) of length T=128.

Per-core layout choices:
  - xs (LSTM inputs) stored [ts=64 (+ ones row), (t, b, d)]: gates/hidden
    live on partitions, batch on the free dim, so the recurrence needs no
    transposes.
  - gate preacts accumulate in PSUM: psum[gate_chunk, batch] =
    Wih.T|bias @ [xs;1] (K=65) + Whh.T @ h (K=128); all matmul operands
    bf16, fp32 accumulation.
  - Gate order per bank is (i, f, o, g) so one sigmoid covers [0:384].
  - h tiles are DMA-xbar-transposed each step into HT [j, t, 2h] for the
    attention-pooling tail.
"""

import sys

for _p in ("/opt/trn_rl_repo", "/root/.axon_site/_ro/trn_rl_repo"):
    if _p not in sys.path:
        sys.path.append(_p)

import numpy as np
import ml_dtypes

import concourse.bass as bass
import concourse.bacc as bacc
import concourse.tile as tile
from concourse import mybir
from concourse.bass_utils import run_bass_kernel_spmd

BF16NP = ml_dtypes.bfloat16
F32 = mybir.dt.float32
F32R = mybir.dt.float32r
BF16 = mybir.dt.bfloat16
AF = mybir.ActivationFunctionType
ALU = mybir.AluOpType

B, T, D, NF = 16, 128, 64, 32
TS, H = 64, 128
NCORES = 8
BLOC = B // NCORES          # 2 samples per core
J = BLOC * D                # 128 sequences per core
R = J * T                   # 16384 (t, b, d) columns
G4 = 4 * H                  # 512 gates per direction
PERM = (0, 1, 3, 2)         # torch gate order (i,f,g,o) -> (i,f,o,g)
NORM_N = D * 2 * H          # 16384 context-norm elements per sample


def emit(tc, ins, outs):
    nc = tc.nc
    XT, WTT, BT = ins["XT"], ins["WTT"], ins["BT"]
    WIH, WHH, ONES = ins["WIH"], ins["WHH"], ins["ONES"]
    DW, DB = ins["DW"], ins["DB"]
    OUT = outs["OUT"]
    from concourse.bass import _add_dep_helper

    with (
        tc.tile_pool(name="consts", bufs=1) as consts,
        tc.tile_pool(name="cpool", bufs=2) as cpool,
        tc.tile_pool(name="sgpool", bufs=2) as sgpool,
        tc.tile_pool(name="small", bufs=2) as small,
    ):
        # ---- constants / weights ----
        wtt = consts.tile([NF, TS], F32)
        nc.sync.dma_start(wtt, WTT)
        bt = consts.tile([TS, 1], F32)
        nc.sync.dma_start(bt, BT)
        wih = consts.tile([TS + 1, 2, G4], BF16)
        nc.sync.dma_start(wih, WIH)
        whh = consts.tile([H, 2, G4], BF16)
        nc.sync.dma_start(whh, WHH)
        # HT: attention layout [j, t, 2h] filled by per-step DMA transposes
        ht = consts.tile([J, T, 2 * H], BF16)

        with (
            tc.tile_pool(name="xs2p", bufs=1) as xs2p,
            tc.tile_pool(name="hslabs", bufs=1) as hslabs,
        ):
            # ---- feature transform (startup phase, fp32 matmuls):
            #      xs2[0:64, (t,b,d)] = tanh(Wt @ X.T + bt)
            xs2 = xs2p.tile([TS + 1, R], BF16)
            nc.sync.dma_start(xs2[TS : TS + 1, :], ONES)
            with (
                tc.tile_pool(name="xtp", bufs=4) as xtp,
                tc.tile_pool(name="tfp", bufs=2, space="PSUM") as tfp,
            ):
                for cc in range(R // 512):
                    xt = xtp.tile([NF, 512], F32, tag="xt")
                    nc.sync.dma_start(xt, XT[:, cc * 512 : (cc + 1) * 512])
                    pz = tfp.tile([TS, 512], F32, tag="pz")
                    nc.tensor.matmul(pz, lhsT=wtt, rhs=xt, start=True, stop=True)
                    nc.scalar.activation(
                        out=xs2[0:TS, cc * 512 : (cc + 1) * 512],
                        in_=pz, func=AF.Tanh, bias=bt, scale=1.0,
                    )
            if "DBG_XS" in outs:
                nc.sync.dma_start(outs["DBG_XS"], xs2[0:TS, 0:512])

            # h history slabs: transpose sources are never recycled
            hsl = [hslabs.tile([H, R], BF16, tag=f"hs{d}", name=f"hs{d}")
                   for d in range(2)]
            h_prev = [None, None]
            c_prev = [None, None]
            for d in range(2):
                h0 = hslabs.tile([H, J], BF16, tag=f"hz{d}", name=f"hz{d}")
                nc.vector.memset(h0, 0.0)
                c0 = cpool.tile([H, J], BF16, tag=f"c{d}")
                nc.vector.memset(c0, 0.0)
                h_prev[d] = h0
                c_prev[d] = c0

            # ---- recurrence ----
            # Gate PSUM groups cover 2 steps: [128, (i,f,o,g), 2*J].
            # Bank layout: chunks 0,1 in bank0 and 2,3 in bank1, so xW
            # matmuls use start=True on each bank's first chunk only.
            with tc.tile_pool(name="gates", bufs=2, space="PSUM") as gates:
                psg = [None, None]
                for t in range(T):
                    u0 = (t % 2) * J
                    if t % 2 == 0:
                        for d in range(2):
                            pg = gates.tile([H, 4, 2 * J], F32, tag=f"g{d}")
                            psg[d] = pg
                            rhs_x = xs2[:, t * J : (t + 2) * J]
                            for c in range(4):
                                nc.tensor.matmul(
                                    pg[:, c, :],
                                    lhsT=wih[:, d, c * H : (c + 1) * H],
                                    rhs=rhs_x, start=(c % 2 == 0), stop=False,
                                )
                    for d in range(2):
                        for c in range(4):
                            nc.tensor.matmul(
                                psg[d][:, c, u0 : u0 + J],
                                lhsT=whh[:, d, c * H : (c + 1) * H],
                                rhs=h_prev[d], start=False,
                                stop=(t % 2 == 1 and c % 2 == 1),
                            )
                    for d in range(2):
                        pg = psg[d]
                        sg = sgpool.tile([H, 3, J], BF16, tag=f"sg{d}")
                        nc.scalar.activation(out=sg, in_=pg[:, 0:3, u0 : u0 + J],
                                             func=AF.Sigmoid)
                        tg = small.tile([H, J], BF16, tag=f"tg{d}")
                        nc.scalar.activation(out=tg, in_=pg[:, 3, u0 : u0 + J],
                                             func=AF.Tanh)
                        c2 = small.tile([H, J], BF16, tag=f"c2{d}")
                        nc.vector.tensor_mul(c2, sg[:, 1, :], c_prev[d])
                        u = small.tile([H, J], BF16, tag=f"u{d}")
                        nc.gpsimd.tensor_mul(u, sg[:, 0, :], tg)
                        cn = cpool.tile([H, J], BF16, tag=f"c{d}")
                        nc.vector.tensor_add(cn, c2, u)
                        tch = small.tile([H, J], BF16, tag=f"tc{d}")
                        nc.scalar.activation(out=tch, in_=cn, func=AF.Tanh)
                        hn = hsl[d][:, t * J : (t + 1) * J]
                        hmul = nc.vector.tensor_mul(hn, sg[:, 2, :], tch)
                        if "DBG_H0" in outs and t == 0 and d == 0:
                            nc.sync.dma_start(outs["DBG_H0"], hn)
                        tr = nc.sync.dma_start_transpose(
                            ht[:, t, d * H : (d + 1) * H], hn
                        )
                        # RAW guard: the xbar transpose must not read hn
                        # before the h write lands (Tile misses this edge)
                        _add_dep_helper(tr.ins, hmul.ins, True,
                                        "xbar transpose reads hn")
                        if "DBG_HT" in outs and t == 1 and d == 1:
                            nc.sync.dma_start(outs["DBG_HT"], ht[:, 0, :])
                        h_prev[d] = hn
                        c_prev[d] = cn

        # ---- tail: attention pooling + context norm ----
        with (
            tc.tile_pool(name="tailp", bufs=1) as tailp,
            tc.tile_pool(name="tailps", bufs=1, space="PSUM") as tailps,
        ):
            htj = ht[:, T - 1, :]  # [J, 2H] last hidden state
            htj_b = bass.AP(
                tensor=htj.tensor, offset=htj.offset,
                ap=[list(htj.ap[0]), [0, T], list(htj.ap[-1])],
            )
            prod = tailp.tile([J, T, 2 * H], BF16)
            nc.vector.tensor_mul(prod, ht, htj_b)
            logits = tailp.tile([J, T], F32)
            nc.vector.tensor_reduce(logits, prod, axis=mybir.AxisListType.X, op=ALU.add)
            mx = tailp.tile([J, 1], F32)
            nc.vector.tensor_reduce(mx, logits, axis=mybir.AxisListType.X, op=ALU.max)
            mxn = tailp.tile([J, 1], F32)
            nc.vector.tensor_scalar_mul(mxn, mx, -1.0)
            ew = tailp.tile([J, T], F32)
            dsum = tailp.tile([J, 1], F32)
            nc.scalar.activation(out=ew, in_=logits, func=AF.Exp, bias=mxn,
                                 scale=1.0, accum_out=dsum)
            rd = tailp.tile([J, 1], F32)
            nc.vector.reciprocal(rd, dsum)
            nc.vector.tensor_scalar_mul(ew, ew, rd)  # softmax weights in place
            ew_b = bass.AP(
                tensor=ew.tensor, offset=ew.offset,
                ap=[list(ew.ap[0]), list(ew.ap[-1]), [0, 2 * H]],
            )
            prod2 = tailp.tile([J, T, 2 * H], BF16, tag="prod")  # reuse slab
            nc.vector.tensor_mul(prod2, ht, ew_b)
            # reduce over t keeping p: view [j, p, t]
            p2v = bass.AP(
                tensor=prod2.tensor, offset=prod2.offset,
                ap=[list(prod2.ap[0]), [1, 2 * H], [2 * H, T]],
            )
            pooled = tailp.tile([J, 2 * H], F32)
            nc.vector.tensor_reduce(pooled, p2v, axis=mybir.AxisListType.X, op=ALU.add)
            if "DBG_LOG" in outs:
                nc.sync.dma_start(outs["DBG_LOG"], logits)
                nc.sync.dma_start(outs["DBG_PO"], pooled)

            # context norm across each sample's (d, 2h) block
            pooled2 = tailp.tile([J, 2 * H], F32)
            nc.scalar.activation(out=pooled2, in_=pooled, func=AF.Square)
            sel = tailp.tile([J, BLOC], F32)
            nc.sync.dma_start(sel, ins["SEL"])
            pstat = tailps.tile([BLOC, 2 * G4], F32, tag="stats")
            nc.tensor.matmul(pstat[:, 0 : 2 * H], lhsT=sel, rhs=pooled,
                             start=True, stop=False)
            nc.tensor.matmul(pstat[:, 2 * H : 4 * H], lhsT=sel, rhs=pooled2,
                             start=False, stop=True)
            s1 = tailp.tile([BLOC, 1], F32)
            nc.vector.tensor_reduce(s1, pstat[:, 0 : 2 * H],
                                    axis=mybir.AxisListType.X, op=ALU.add)
            s2 = tailp.tile([BLOC, 1], F32)
            nc.vector.tensor_reduce(s2, pstat[:, 2 * H : 4 * H],
                                    axis=mybir.AxisListType.X, op=ALU.add)
            stats2 = tailp.tile([BLOC, 2], F32)
            nc.scalar.mul(stats2[:, 0:1], s1, 1.0 / NORM_N)      # mean
            q = tailp.tile([BLOC, 1], F32)
            nc.vector.tensor_mul(q, s1, stats2[:, 0:1])          # sum*mean
            v = tailp.tile([BLOC, 1], F32)
            nc.vector.tensor_tensor(v, s2, q, op=ALU.subtract)
            sd = tailp.tile([BLOC, 1], F32)
            nc.scalar.activation(out=sd, in_=v, func=AF.Sqrt,
                                 scale=1.0 / (NORM_N - 1))
            nc.vector.reciprocal(stats2[:, 1:2], sd)             # rstd
            selt = tailp.tile([BLOC, J], F32)
            nc.sync.dma_start(selt, ins["SELT"])
            pmb = tailps.tile([J, 2], F32, tag="mb")
            nc.tensor.matmul(pmb, lhsT=selt, rhs=stats2, start=True, stop=True)
            mb = tailp.tile([J, 2], F32)
            nc.vector.tensor_copy(mb, pmb)
            dwt = tailp.tile([J, 2 * H], F32)
            nc.sync.dma_start(dwt[0:D, :], DW)
            nc.sync.dma_start(dwt[D:J, :], DW)
            dbt = tailp.tile([J, 2 * H], F32)
            nc.sync.dma_start(dbt[0:D, :], DB)
            nc.sync.dma_start(dbt[D:J, :], DB)
            t1 = tailp.tile([J, 2 * H], F32)
            nc.vector.tensor_scalar(t1, pooled, mb[:, 0:1], mb[:, 1:2],
                                    op0=ALU.subtract, op1=ALU.mult)
            t2 = tailp.tile([J, 2 * H], F32)
            nc.vector.tensor_mul(t2, t1, dwt)
            t3 = tailp.tile([J, 2 * H], F32)
            nc.vector.tensor_add(t3, t2, dbt)
            nc.sync.dma_start(OUT, t3)


def build_program():
    nc = bacc.Bacc("TRN2", target_bir_lowering=False, debug=False)
    ins = {
        "XT": nc.dram_tensor("XT", [NF, R], F32, kind="ExternalInput").ap(),
        "WTT": nc.dram_tensor("WTT", [NF, TS], F32, kind="ExternalInput").ap(),
        "BT": nc.dram_tensor("BT", [TS, 1], F32, kind="ExternalInput").ap(),
        "WIH": nc.dram_tensor("WIH", [TS + 1, 2, G4], BF16, kind="ExternalInput").ap(),
        "WHH": nc.dram_tensor("WHH", [H, 2, G4], BF16, kind="ExternalInput").ap(),
        "ONES": nc.dram_tensor("ONES", [1, R], BF16, kind="ExternalInput").ap(),
        "DW": nc.dram_tensor("DW", [D, 2 * H], F32, kind="ExternalInput").ap(),
        "SEL": nc.dram_tensor("SEL", [J, BLOC], F32, kind="ExternalInput").ap(),
        "SELT": nc.dram_tensor("SELT", [BLOC, J], F32, kind="ExternalInput").ap(),
        "DB": nc.dram_tensor("DB", [D, 2 * H], F32, kind="ExternalInput").ap(),
    }
    outs = {
        "OUT": nc.dram_tensor("OUT", [J, 2 * H], F32, kind="ExternalOutput").ap(),
    }
    with tile.TileContext(nc) as tc:
        emit(tc, ins, outs)
    nc.compile()
    return nc


def _prep_dir(Wih, Whh, bih, bhh):
    wihT = Wih.T.reshape(TS, 4, H)[:, PERM, :].reshape(TS, G4)
    biasr = (bih + bhh).reshape(4, H)[PERM, :].reshape(G4)
    wih65 = np.concatenate([wihT, biasr[None, :]], axis=0).astype(BF16NP)
    whhT = Whh.T.reshape(H, 4, H)[:, PERM, :].reshape(H, G4).astype(BF16NP)
    return wih65, whhT


def prep_inputs(X, W_t, b_t, Wih_f, Whh_f, bih_f, bhh_f,
                Wih_b, Whh_b, bih_b, bhh_b, diag_w, diag_b):
    wih_f, whh_f = _prep_dir(Wih_f, Whh_f, bih_f, bhh_f)
    wih_b, whh_b = _prep_dir(Wih_b, Whh_b, bih_b, bhh_b)
    shared = {
        "WTT": np.ascontiguousarray(W_t.T, dtype=np.float32),
        "BT": np.ascontiguousarray(b_t.reshape(TS, 1), dtype=np.float32),
        "WIH": np.ascontiguousarray(np.stack([wih_f, wih_b], axis=1)),
        "WHH": np.ascontiguousarray(np.stack([whh_f, whh_b], axis=1)),
        "ONES": np.ones((1, R), dtype=BF16NP),
        "SEL": np.kron(np.eye(BLOC, dtype=np.float32), np.ones((D, 1), np.float32)),
        "SELT": np.kron(np.eye(BLOC, dtype=np.float32), np.ones((1, D), np.float32)),
        "DW": np.ascontiguousarray(diag_w.reshape(D, 2 * H), dtype=np.float32),
        "DB": np.ascontiguousarray(diag_b.reshape(D, 2 * H), dtype=np.float32),
    }
    in_maps = []
    for i in range(NCORES):
        xt = np.ascontiguousarray(
            X[i * BLOC : (i + 1) * BLOC].transpose(3, 1, 0, 2).reshape(NF, R),
            dtype=np.float32,
        )
        m = {"XT": xt}
        m.update(shared)
        in_maps.append(m)
    return in_maps


def kernel(**inputs):
    inputs = {k: np.asarray(v, dtype=np.float32) for k, v in inputs.items()}
    in_maps = prep_inputs(**inputs)
    nc = build_program()
    res = run_bass_kernel_spmd(nc, in_maps, list(range(NCORES)))
    out = np.concatenate(
        [res.results[i]["OUT"].reshape(BLOC, D, 2 * H) for i in range(NCORES)],
        axis=0,
    )
    return np.ascontiguousarray(out, dtype=np.float32)


if __name__ == "__main__":
    nc = build_program()
    print("program built ok")


# revision 19
# speedup vs baseline: 2.0395x; 1.1271x over previous
"""Trainium2 Bass kernel for nn_ContextEncoder.

Pipeline (per sample b): feature transform tanh(X @ W_t.T + b_t), a
"bidirectional" LSTM where both directions run forward (matching the
reference), attention pooling against the last hidden state, and a
context norm over the flattened (d, 2h) vector.

Sharding: data-parallel over b (16 samples -> 2 per core on 8 cores).
Each core runs 128 independent sequences (2 b x 64 d) of length T=128.

Per-core layout choices:
  - xs (LSTM inputs) stored [ts=64 (+ ones row), (t, b, d)]: gates/hidden
    live on partitions, batch on the free dim, so the recurrence needs no
    transposes.
  - gate preacts accumulate in PSUM: psum[gate_chunk, batch] =
    Wih.T|bias @ [xs;1] (K=65) + Whh.T @ h (K=128); all matmul operands
    bf16, fp32 accumulation.
  - Gate order per bank is (i, f, o, g) so one sigmoid covers [0:384].
  - h tiles are DMA-xbar-transposed each step into HT [j, t, 2h] for the
    attention-pooling tail.
"""

import sys

for _p in ("/opt/trn_rl_repo", "/root/.axon_site/_ro/trn_rl_repo"):
    if _p not in sys.path:
        sys.path.append(_p)

import numpy as np
import ml_dtypes

import concourse.bass as bass
import concourse.bacc as bacc
import concourse.tile as tile
from concourse import mybir
from concourse.bass_utils import run_bass_kernel_spmd

BF16NP = ml_dtypes.bfloat16
F32 = mybir.dt.float32
F32R = mybir.dt.float32r
BF16 = mybir.dt.bfloat16
AF = mybir.ActivationFunctionType
ALU = mybir.AluOpType

B, T, D, NF = 16, 128, 64, 32
TS, H = 64, 128
NCORES = 8
BLOC = B // NCORES          # 2 samples per core
J = BLOC * D                # 128 sequences per core
R = J * T                   # 16384 (t, b, d) columns
G4 = 4 * H                  # 512 gates per direction
PERM = (0, 1, 3, 2)         # torch gate order (i,f,g,o) -> (i,f,o,g)
NORM_N = D * 2 * H          # 16384 context-norm elements per sample


def emit(tc, ins, outs):
    nc = tc.nc
    XT, WTT, BT = ins["XT"], ins["WTT"], ins["BT"]
    WIH, WHH, ONES = ins["WIH"], ins["WHH"], ins["ONES"]
    DW, DB = ins["DW"], ins["DB"]
    OUT = outs["OUT"]
    from concourse.bass import _add_dep_helper

    with (
        tc.tile_pool(name="consts", bufs=1) as consts,
        tc.tile_pool(name="cpool", bufs=2) as cpool,
        tc.tile_pool(name="sgpool", bufs=2) as sgpool,
        tc.tile_pool(name="small", bufs=2) as small,
    ):
        # ---- constants / weights ----
        wtt = consts.tile([NF, TS], F32)
        nc.sync.dma_start(wtt, WTT)
        bt = consts.tile([TS, 1], F32)
        nc.sync.dma_start(bt, BT)
        wih = consts.tile([TS + 1, 2, G4], BF16)
        nc.sync.dma_start(wih, WIH)
        whh = consts.tile([H, 2, G4], BF16)
        nc.sync.dma_start(whh, WHH)
        # HT: attention layout [j, t, 2h] filled by per-step DMA transposes
        ht = consts.tile([J, T, 2 * H], BF16)

        with (
            tc.tile_pool(name="xs2p", bufs=1) as xs2p,
            tc.tile_pool(name="hslabs", bufs=1) as hslabs,
        ):
            # ---- feature transform (startup phase, fp32 matmuls):
            #      xs2[0:64, (t,b,d)] = tanh(Wt @ X.T + bt)
            xs2 = xs2p.tile([TS + 1, R], BF16)
            nc.sync.dma_start(xs2[TS : TS + 1, :], ONES)
            with (
                tc.tile_pool(name="xtp", bufs=4) as xtp,
                tc.tile_pool(name="tfp", bufs=2, space="PSUM") as tfp,
            ):
                for cc in range(R // 512):
                    xt = xtp.tile([NF, 512], F32, tag="xt")
                    nc.sync.dma_start(xt, XT[:, cc * 512 : (cc + 1) * 512])
                    pz = tfp.tile([TS, 512], F32, tag="pz")
                    nc.tensor.matmul(pz, lhsT=wtt, rhs=xt, start=True, stop=True)
                    nc.scalar.activation(
                        out=xs2[0:TS, cc * 512 : (cc + 1) * 512],
                        in_=pz, func=AF.Tanh, bias=bt, scale=1.0,
                    )
            if "DBG_XS" in outs:
                nc.sync.dma_start(outs["DBG_XS"], xs2[0:TS, 0:512])

            # h history slabs: transpose sources are never recycled
            hsl = [hslabs.tile([H, R], BF16, tag=f"hs{d}", name=f"hs{d}")
                   for d in range(2)]
            h_prev = [None, None]
            c_prev = [None, None]
            for d in range(2):
                h0 = hslabs.tile([H, J], BF16, tag=f"hz{d}", name=f"hz{d}")
                nc.vector.memset(h0, 0.0)
                c0 = cpool.tile([H, J], BF16, tag=f"c{d}")
                nc.vector.memset(c0, 0.0)
                h_prev[d] = h0
                c_prev[d] = c0

            # ---- recurrence ----
            # Gate PSUM groups cover 2 steps: [128, (i,f,o,g), 2*J].
            # Bank layout: chunks 0,1 in bank0 and 2,3 in bank1, so xW
            # matmuls use start=True on each bank's first chunk only.
            with tc.tile_pool(name="gates", bufs=2, space="PSUM") as gates:
                psg = [None, None]
                for t in range(T):
                    u0 = (t % 2) * J
                    if t % 2 == 0:
                        for d in range(2):
                            pg = gates.tile([H, 4, 2 * J], F32, tag=f"g{d}")
                            psg[d] = pg
                            rhs_x = xs2[:, t * J : (t + 2) * J]
                            for c in range(4):
                                nc.tensor.matmul(
                                    pg[:, c, :],
                                    lhsT=wih[:, d, c * H : (c + 1) * H],
                                    rhs=rhs_x, start=(c % 2 == 0), stop=False,
                                )
                    for d in range(2):
                        for c in range(4):
                            nc.tensor.matmul(
                                psg[d][:, c, u0 : u0 + J],
                                lhsT=whh[:, d, c * H : (c + 1) * H],
                                rhs=h_prev[d], start=False,
                                stop=(t % 2 == 1 and c % 2 == 1),
                            )
                    for d in range(2):
                        pg = psg[d]
                        sg = sgpool.tile([H, 3, J], BF16, tag=f"sg{d}")
                        nc.scalar.activation(out=sg, in_=pg[:, 0:3, u0 : u0 + J],
                                             func=AF.Sigmoid)
                        tg = small.tile([H, J], BF16, tag=f"tg{d}")
                        nc.scalar.activation(out=tg, in_=pg[:, 3, u0 : u0 + J],
                                             func=AF.Tanh)
                        c2 = small.tile([H, J], BF16, tag=f"c2{d}")
                        nc.vector.tensor_mul(c2, sg[:, 1, :], c_prev[d])
                        u = small.tile([H, J], BF16, tag=f"u{d}")
                        nc.vector.tensor_mul(u, sg[:, 0, :], tg)
                        cn = cpool.tile([H, J], BF16, tag=f"c{d}")
                        nc.vector.tensor_add(cn, c2, u)
                        tch = small.tile([H, J], BF16, tag=f"tc{d}")
                        nc.scalar.activation(out=tch, in_=cn, func=AF.Tanh)
                        hn = hsl[d][:, t * J : (t + 1) * J]
                        hmul = nc.vector.tensor_mul(hn, sg[:, 2, :], tch)
                        if "DBG_H0" in outs and t == 0 and d == 0:
                            nc.sync.dma_start(outs["DBG_H0"], hn)
                        tr = nc.sync.dma_start_transpose(
                            ht[:, t, d * H : (d + 1) * H], hn
                        )
                        # RAW guard: the xbar transpose must not read hn
                        # before the h write lands (Tile misses this edge)
                        _add_dep_helper(tr.ins, hmul.ins, True,
                                        "xbar transpose reads hn")
                        if "DBG_HT" in outs and t == 1 and d == 1:
                            nc.sync.dma_start(outs["DBG_HT"], ht[:, 0, :])
                        h_prev[d] = hn
                        c_prev[d] = cn

        # ---- tail: attention pooling + context norm ----
        with (
            tc.tile_pool(name="tailp", bufs=1) as tailp,
            tc.tile_pool(name="tailps", bufs=1, space="PSUM") as tailps,
        ):
            htj = ht[:, T - 1, :]  # [J, 2H] last hidden state
            htj_b = bass.AP(
                tensor=htj.tensor, offset=htj.offset,
                ap=[list(htj.ap[0]), [0, T], list(htj.ap[-1])],
            )
            prod = tailp.tile([J, T, 2 * H], BF16)
            nc.vector.tensor_mul(prod, ht, htj_b)
            # pairwise-tree sum over p (innermost), in place: bf16 to 64, then fp32
            w = 2 * H
            while w > 64:
                w //= 2
                nc.vector.tensor_add(prod[:, :, 0:w], prod[:, :, 0:w],
                                     prod[:, :, w : 2 * w])
            lvf = tailp.tile([J, T, 64], F32, tag="ltrf")
            nc.vector.tensor_copy(lvf, prod[:, :, 0:64])
            while w > 1:
                w //= 2
                nc.vector.tensor_add(lvf[:, :, 0:w], lvf[:, :, 0:w],
                                     lvf[:, :, w : 2 * w])
            logits = lvf[:, :, 0:1].rearrange("j t one -> j (t one)")
            mx = tailp.tile([J, 1], F32)
            nc.vector.tensor_reduce(mx, logits, axis=mybir.AxisListType.X, op=ALU.max)
            mxn = tailp.tile([J, 1], F32)
            nc.vector.tensor_scalar_mul(mxn, mx, -1.0)
            ew = tailp.tile([J, T], F32)
            dsum = tailp.tile([J, 1], F32)
            nc.scalar.activation(out=ew, in_=logits, func=AF.Exp, bias=mxn,
                                 scale=1.0, accum_out=dsum)
            rd = tailp.tile([J, 1], F32)
            nc.vector.reciprocal(rd, dsum)
            nc.vector.tensor_scalar_mul(ew, ew, rd)  # softmax weights in place
            ewb = tailp.tile([J, T], BF16)
            nc.vector.tensor_copy(ewb, ew)
            ew_b = bass.AP(
                tensor=ewb.tensor, offset=ewb.offset,
                ap=[list(ewb.ap[0]), list(ewb.ap[-1]), [0, 2 * H]],
            )
            prod2 = tailp.tile([J, T, 2 * H], BF16, tag="prod")  # reuse slab
            nc.vector.tensor_mul(prod2, ht, ew_b)
            # pairwise-tree sum over t (outer free dim), in place: bf16 to 16, then fp32
            wt = T
            while wt > 16:
                wt //= 2
                nc.vector.tensor_add(prod2[:, 0:wt, :], prod2[:, 0:wt, :],
                                     prod2[:, wt : 2 * wt, :])
            lv2f = tailp.tile([J, 16, 2 * H], F32, tag="ptrf")
            nc.vector.tensor_copy(lv2f, prod2[:, 0:16, :])
            while wt > 1:
                wt //= 2
                nc.vector.tensor_add(lv2f[:, 0:wt, :], lv2f[:, 0:wt, :],
                                     lv2f[:, wt : 2 * wt, :])
            pooled = lv2f[:, 0:1, :].rearrange("j one p -> j (one p)")
            if "DBG_LOG" in outs:
                nc.sync.dma_start(outs["DBG_LOG"], logits)
                nc.sync.dma_start(outs["DBG_PO"], pooled)

            # context norm across each sample's (d, 2h) block
            pooled2 = tailp.tile([J, 2 * H], F32)
            nc.scalar.activation(out=pooled2, in_=pooled, func=AF.Square)
            sel = tailp.tile([J, BLOC], F32)
            nc.sync.dma_start(sel, ins["SEL"])
            pstat = tailps.tile([BLOC, 2 * G4], F32, tag="stats")
            nc.tensor.matmul(pstat[:, 0 : 2 * H], lhsT=sel, rhs=pooled,
                             start=True, stop=False)
            nc.tensor.matmul(pstat[:, 2 * H : 4 * H], lhsT=sel, rhs=pooled2,
                             start=False, stop=True)
            s1 = tailp.tile([BLOC, 1], F32)
            nc.vector.tensor_reduce(s1, pstat[:, 0 : 2 * H],
                                    axis=mybir.AxisListType.X, op=ALU.add)
            s2 = tailp.tile([BLOC, 1], F32)
            nc.vector.tensor_reduce(s2, pstat[:, 2 * H : 4 * H],
                                    axis=mybir.AxisListType.X, op=ALU.add)
            stats2 = tailp.tile([BLOC, 2], F32)
            nc.scalar.mul(stats2[:, 0:1], s1, 1.0 / NORM_N)      # mean
            q = tailp.tile([BLOC, 1], F32)
            nc.vector.tensor_mul(q, s1, stats2[:, 0:1])          # sum*mean
            v = tailp.tile([BLOC, 1], F32)
            nc.vector.tensor_tensor(v, s2, q, op=ALU.subtract)
            sd = tailp.tile([BLOC, 1], F32)
            nc.scalar.activation(out=sd, in_=v, func=AF.Sqrt,
                                 scale=1.0 / (NORM_N - 1))
            nc.vector.reciprocal(stats2[:, 1:2], sd)             # rstd
            selt = tailp.tile([BLOC, J], F32)
            nc.sync.dma_start(selt, ins["SELT"])
            pmb = tailps.tile([J, 2], F32, tag="mb")
            nc.tensor.matmul(pmb, lhsT=selt, rhs=stats2, start=True, stop=True)
            mb = tailp.tile([J, 2], F32)
            nc.vector.tensor_copy(mb, pmb)
            dwt = tailp.tile([J, 2 * H], F32)
            nc.sync.dma_start(dwt[0:D, :], DW)
            nc.sync.dma_start(dwt[D:J, :], DW)
            dbt = tailp.tile([J, 2 * H], F32)
            nc.sync.dma_start(dbt[0:D, :], DB)
            nc.sync.dma_start(dbt[D:J, :], DB)
            t1 = tailp.tile([J, 2 * H], F32)
            nc.vector.tensor_scalar(t1, pooled, mb[:, 0:1], mb[:, 1:2],
                                    op0=ALU.subtract, op1=ALU.mult)
            t2 = tailp.tile([J, 2 * H], F32)
            nc.vector.tensor_mul(t2, t1, dwt)
            t3 = tailp.tile([J, 2 * H], F32)
            nc.vector.tensor_add(t3, t2, dbt)
            nc.sync.dma_start(OUT, t3)


def build_program():
    nc = bacc.Bacc("TRN2", target_bir_lowering=False, debug=False)
    ins = {
        "XT": nc.dram_tensor("XT", [NF, R], F32, kind="ExternalInput").ap(),
        "WTT": nc.dram_tensor("WTT", [NF, TS], F32, kind="ExternalInput").ap(),
        "BT": nc.dram_tensor("BT", [TS, 1], F32, kind="ExternalInput").ap(),
        "WIH": nc.dram_tensor("WIH", [TS + 1, 2, G4], BF16, kind="ExternalInput").ap(),
        "WHH": nc.dram_tensor("WHH", [H, 2, G4], BF16, kind="ExternalInput").ap(),
        "ONES": nc.dram_tensor("ONES", [1, R], BF16, kind="ExternalInput").ap(),
        "DW": nc.dram_tensor("DW", [D, 2 * H], F32, kind="ExternalInput").ap(),
        "SEL": nc.dram_tensor("SEL", [J, BLOC], F32, kind="ExternalInput").ap(),
        "SELT": nc.dram_tensor("SELT", [BLOC, J], F32, kind="ExternalInput").ap(),
        "DB": nc.dram_tensor("DB", [D, 2 * H], F32, kind="ExternalInput").ap(),
    }
    outs = {
        "OUT": nc.dram_tensor("OUT", [J, 2 * H], F32, kind="ExternalOutput").ap(),
    }
    with tile.TileContext(nc) as tc:
        emit(tc, ins, outs)
    nc.compile()
    return nc


def _prep_dir(Wih, Whh, bih, bhh):
    wihT = Wih.T.reshape(TS, 4, H)[:, PERM, :].reshape(TS, G4)
    biasr = (bih + bhh).reshape(4, H)[PERM, :].reshape(G4)
    wih65 = np.concatenate([wihT, biasr[None, :]], axis=0).astype(BF16NP)
    whhT = Whh.T.reshape(H, 4, H)[:, PERM, :].reshape(H, G4).astype(BF16NP)
    return wih65, whhT


def prep_inputs(X, W_t, b_t, Wih_f, Whh_f, bih_f, bhh_f,
                Wih_b, Whh_b, bih_b, bhh_b, diag_w, diag_b):
    wih_f, whh_f = _prep_dir(Wih_f, Whh_f, bih_f, bhh_f)
    wih_b, whh_b = _prep_dir(Wih_b, Whh_b, bih_b, bhh_b)
    shared = {
        "WTT": np.ascontiguousarray(W_t.T, dtype=np.float32),
        "BT": np.ascontiguousarray(b_t.reshape(TS, 1), dtype=np.float32),
        "WIH": np.ascontiguousarray(np.stack([wih_f, wih_b], axis=1)),
        "WHH": np.ascontiguousarray(np.stack([whh_f, whh_b], axis=1)),
        "ONES": np.ones((1, R), dtype=BF16NP),
        "SEL": np.kron(np.eye(BLOC, dtype=np.float32), np.ones((D, 1), np.float32)),
        "SELT": np.kron(np.eye(BLOC, dtype=np.float32), np.ones((1, D), np.float32)),
        "DW": np.ascontiguousarray(diag_w.reshape(D, 2 * H), dtype=np.float32),
        "DB": np.ascontiguousarray(diag_b.reshape(D, 2 * H), dtype=np.float32),
    }
    in_maps = []
    for i in range(NCORES):
        xt = np.ascontiguousarray(
            X[i * BLOC : (i + 1) * BLOC].transpose(3, 1, 0, 2).reshape(NF, R),
            dtype=np.float32,
        )
        m = {"XT": xt}
        m.update(shared)
        in_maps.append(m)
    return in_maps


def kernel(**inputs):
    inputs = {k: np.asarray(v, dtype=np.float32) for k, v in inputs.items()}
    in_maps = prep_inputs(**inputs)
    nc = build_program()
    res = run_bass_kernel_spmd(nc, in_maps, list(range(NCORES)))
    out = np.concatenate(
        [res.results[i]["OUT"].reshape(BLOC, D, 2 * H) for i in range(NCORES)],
        axis=0,
    )
    return np.ascontiguousarray(out, dtype=np.float32)


if __name__ == "__main__":
    nc = build_program()
    print("program built ok")


# revision 23
# speedup vs baseline: 2.2861x; 1.1209x over previous
"""Trainium2 Bass kernel for nn_ContextEncoder.

Pipeline (per sample b): feature transform tanh(X @ W_t.T + b_t), a
"bidirectional" LSTM where both directions run forward (matching the
reference), attention pooling against the last hidden state, and a
context norm over the flattened (d, 2h) vector.

Sharding: data-parallel over b (16 samples -> 2 per core on 8 cores).
Each core runs 128 independent sequences (2 b x 64 d) of length T=128.

Per-core layout choices:
  - xs (LSTM inputs) stored [ts=64 (+ ones row), (t, b, d)]: gates/hidden
    live on partitions, batch on the free dim, so the recurrence needs no
    transposes.
  - gate preacts accumulate in PSUM: psum[gate_chunk, batch] =
    Wih.T|bias @ [xs;1] (K=65) + Whh.T @ h (K=128); all matmul operands
    bf16, fp32 accumulation.
  - Gate order per bank is (i, f, o, g) so one sigmoid covers [0:384].
  - h tiles are DMA-xbar-transposed each step into HT [j, t, 2h] for the
    attention-pooling tail.
"""

import sys

for _p in ("/opt/trn_rl_repo", "/root/.axon_site/_ro/trn_rl_repo"):
    if _p not in sys.path:
        sys.path.append(_p)

import numpy as np
import ml_dtypes

import concourse.bass as bass
import concourse.bacc as bacc
import concourse.tile as tile
from concourse import mybir
from concourse.bass_utils import run_bass_kernel_spmd

BF16NP = ml_dtypes.bfloat16
F32 = mybir.dt.float32
F32R = mybir.dt.float32r
BF16 = mybir.dt.bfloat16
AF = mybir.ActivationFunctionType
ALU = mybir.AluOpType

B, T, D, NF = 16, 128, 64, 32
TS, H = 64, 128
NCORES = 8
BLOC = B // NCORES          # 2 samples per core
J = BLOC * D                # 128 sequences per core
R = J * T                   # 16384 (t, b, d) columns
G4 = 4 * H                  # 512 gates per direction
PERM = (0, 1, 3, 2)         # torch gate order (i,f,g,o) -> (i,f,o,g)
NORM_N = D * 2 * H          # 16384 context-norm elements per sample


def pg_slice(pg, u0):
    return pg[:, :, u0 : u0 + J]


def emit(tc, ins, outs):
    nc = tc.nc
    XT, WTT, BT = ins["XT"], ins["WTT"], ins["BT"]
    WIH, WHH, ONES = ins["WIH"], ins["WHH"], ins["ONES"]
    DW, DB = ins["DW"], ins["DB"]
    OUT = outs["OUT"]
    from concourse.bass import _add_dep_helper

    with (
        tc.tile_pool(name="consts", bufs=1) as consts,
        tc.tile_pool(name="cpool", bufs=2) as cpool,
        tc.tile_pool(name="sgpool", bufs=2) as sgpool,
        tc.tile_pool(name="small", bufs=2) as small,
    ):
        # ---- constants / weights ----
        wtt = consts.tile([NF, TS], F32)
        nc.sync.dma_start(wtt, WTT)
        bt = consts.tile([TS, 1], F32)
        nc.sync.dma_start(bt, BT)
        wih = consts.tile([TS + 1, 2, G4], BF16)
        nc.sync.dma_start(wih, WIH)
        whh = consts.tile([H, 2, G4], BF16)
        nc.sync.dma_start(whh, WHH)
        # HT: attention layout [j, t, 2h] filled by per-step DMA transposes
        ht = consts.tile([J, T, 2 * H], BF16)

        with (
            tc.tile_pool(name="xs2p", bufs=1) as xs2p,
            tc.tile_pool(name="hslabs", bufs=1) as hslabs,
        ):
            # ---- feature transform (startup phase, fp32 matmuls):
            #      xs2[0:64, (t,b,d)] = tanh(Wt @ X.T + bt)
            xs2 = xs2p.tile([TS + 1, R], BF16)
            nc.sync.dma_start(xs2[TS : TS + 1, :], ONES)
            with (
                tc.tile_pool(name="xtp", bufs=4) as xtp,
                tc.tile_pool(name="tfp", bufs=2, space="PSUM") as tfp,
            ):
                for cc in range(R // 512):
                    xt = xtp.tile([NF, 512], F32, tag="xt")
                    nc.sync.dma_start(xt, XT[:, cc * 512 : (cc + 1) * 512])
                    pz = tfp.tile([TS, 512], F32, tag="pz")
                    nc.tensor.matmul(pz, lhsT=wtt, rhs=xt, start=True, stop=True)
                    nc.scalar.activation(
                        out=xs2[0:TS, cc * 512 : (cc + 1) * 512],
                        in_=pz, func=AF.Tanh, bias=bt, scale=1.0,
                    )
            if "DBG_XS" in outs:
                nc.sync.dma_start(outs["DBG_XS"], xs2[0:TS, 0:512])

            # h history slabs: transpose sources are never recycled
            hsl = [hslabs.tile([H, R], BF16, tag=f"hs{d}", name=f"hs{d}")
                   for d in range(2)]
            h_prev = [None, None]
            c_prev = [None, None]
            for d in range(2):
                h0 = hslabs.tile([H, J], BF16, tag=f"hz{d}", name=f"hz{d}")
                nc.vector.memset(h0, 0.0)
                c0 = cpool.tile([H, J], BF16, tag=f"c{d}")
                nc.vector.memset(c0, 0.0)
                h_prev[d] = h0
                c_prev[d] = c0

            # ---- recurrence ----
            # Gate PSUM groups cover 2 steps: [128, (i,f,o,g), 2*J].
            # Bank layout: chunks 0,1 in bank0 and 2,3 in bank1, so xW
            # matmuls use start=True on each bank's first chunk only.
            with tc.tile_pool(name="gates", bufs=2, space="PSUM") as gates:
                psg = [None, None]
                for t in range(T):
                    u0 = (t % 2) * J
                    if t % 2 == 0:
                        for d in range(2):
                            pg = gates.tile([H, 4, 2 * J], F32, tag=f"g{d}")
                            psg[d] = pg
                            rhs_x = xs2[:, t * J : (t + 2) * J]
                            for c in range(4):
                                nc.tensor.matmul(
                                    pg[:, c, :],
                                    lhsT=wih[:, d, c * H : (c + 1) * H],
                                    rhs=rhs_x, start=(c % 2 == 0), stop=False,
                                )
                    for d in range(2):
                        for c in range(4):
                            nc.tensor.matmul(
                                psg[d][:, c, u0 : u0 + J],
                                lhsT=whh[:, d, c * H : (c + 1) * H],
                                rhs=h_prev[d], start=False,
                                stop=(t % 2 == 1 and c % 2 == 1),
                            )
                    sg4, tg_, c2_, u_, cn_, tc_ = {}, {}, {}, {}, {}, {}
                    for d in range(2):
                        sg4[d] = sgpool.tile([H, 4, J], BF16, tag=f"sg{d}",
                                             name=f"sg{d}")
                        nc.scalar.activation(out=sg4[d], in_=pg_slice(psg[d], u0),
                                             func=AF.Sigmoid)
                    for d in range(2):
                        tg_[d] = small.tile([H, J], BF16, tag=f"tg{d}",
                                            name=f"tg{d}")
                        nc.vector.tensor_scalar(tg_[d], sg4[d][:, 3, :], 2.0, -1.0,
                                                op0=ALU.mult, op1=ALU.add)
                    for d in range(2):
                        c2_[d] = small.tile([H, J], BF16, tag=f"c2{d}",
                                            name=f"c2{d}")
                        nc.vector.tensor_mul(c2_[d], sg4[d][:, 1, :], c_prev[d])
                    for d in range(2):
                        u_[d] = small.tile([H, J], BF16, tag=f"u{d}", name=f"u{d}")
                        nc.vector.tensor_mul(u_[d], sg4[d][:, 0, :], tg_[d])
                    for d in range(2):
                        cn_[d] = cpool.tile([H, J], BF16, tag=f"c{d}",
                                            name=f"cn{d}")
                        nc.vector.tensor_add(cn_[d], c2_[d], u_[d])
                    for d in range(2):
                        tc_[d] = small.tile([H, J], BF16, tag=f"tc{d}",
                                            name=f"tc{d}")
                        nc.scalar.activation(out=tc_[d], in_=cn_[d], func=AF.Tanh)
                    for d in range(2):
                        hn = hsl[d][:, t * J : (t + 1) * J]
                        hmul = nc.vector.tensor_mul(hn, sg4[d][:, 2, :], tc_[d])
                        if "DBG_H0" in outs and t == 0 and d == 0:
                            nc.sync.dma_start(outs["DBG_H0"], hn)
                        tr = nc.sync.dma_start_transpose(
                            ht[:, t, d * H : (d + 1) * H], hn
                        )
                        # RAW guard: the xbar transpose must not read hn
                        # before the h write lands (Tile misses this edge)
                        _add_dep_helper(tr.ins, hmul.ins, True,
                                        "xbar transpose reads hn")
                        if "DBG_HT" in outs and t == 1 and d == 1:
                            nc.sync.dma_start(outs["DBG_HT"], ht[:, 0, :])
                        h_prev[d] = hn
                        c_prev[d] = cn_[d]

        # ---- tail: attention pooling + context norm ----
        with (
            tc.tile_pool(name="tailp", bufs=1) as tailp,
            tc.tile_pool(name="tailps", bufs=1, space="PSUM") as tailps,
        ):
            htj = ht[:, T - 1, :]  # [J, 2H] last hidden state
            htj_b = bass.AP(
                tensor=htj.tensor, offset=htj.offset,
                ap=[list(htj.ap[0]), [0, T], list(htj.ap[-1])],
            )
            prod = tailp.tile([J, T, 2 * H], BF16)
            nc.vector.tensor_mul(prod, ht, htj_b)
            # pairwise-tree sum over p: bf16 levels ping-pong {pp0, prod}, then fp32
            pp0 = tailp.tile([J, T, 128], BF16)
            nc.vector.tensor_add(pp0, prod[:, :, 0:128], prod[:, :, 128:256])
            nc.vector.tensor_add(prod[:, :, 0:64], pp0[:, :, 0:64], pp0[:, :, 64:128])
            nc.vector.tensor_add(pp0[:, :, 0:32], prod[:, :, 0:32], prod[:, :, 32:64])
            ltrf = tailp.tile([J, T, 16], F32)
            nc.vector.tensor_add(ltrf, pp0[:, :, 0:16], pp0[:, :, 16:32])
            w = 16
            while w > 1:
                w //= 2
                nc.vector.tensor_add(ltrf[:, :, 0:w], ltrf[:, :, 0:w],
                                     ltrf[:, :, w : 2 * w])
            logits = ltrf[:, :, 0:1].rearrange("j t one -> j (t one)")
            mx = tailp.tile([J, 1], F32)
            nc.vector.tensor_reduce(mx, logits, axis=mybir.AxisListType.X, op=ALU.max)
            mxn = tailp.tile([J, 1], F32)
            nc.vector.tensor_scalar_mul(mxn, mx, -1.0)
            ew = tailp.tile([J, T], F32)
            dsum = tailp.tile([J, 1], F32)
            nc.scalar.activation(out=ew, in_=logits, func=AF.Exp, bias=mxn,
                                 scale=1.0, accum_out=dsum)
            rd = tailp.tile([J, 1], F32)
            nc.vector.reciprocal(rd, dsum)
            nc.vector.tensor_scalar_mul(ew, ew, rd)  # softmax weights in place
            ewb = tailp.tile([J, T], BF16)
            nc.vector.tensor_copy(ewb, ew)
            ew_b = bass.AP(
                tensor=ewb.tensor, offset=ewb.offset,
                ap=[list(ewb.ap[0]), list(ewb.ap[-1]), [0, 2 * H]],
            )
            prod2 = tailp.tile([J, T, 2 * H], BF16, tag="prod")  # reuse slab
            nc.vector.tensor_mul(prod2, ht, ew_b)
            # pairwise-tree sum over t: bf16 levels ping-pong {pp0-view, prod2}
            qq = pp0.rearrange("j a b -> j (a b)").rearrange(
                "j (a b) -> j a b", a=64)
            nc.vector.tensor_add(qq, prod2[:, 0:64, :], prod2[:, 64:128, :])
            nc.vector.tensor_add(prod2[:, 0:32, :], qq[:, 0:32, :], qq[:, 32:64, :])
            nc.vector.tensor_add(qq[:, 0:16, :], prod2[:, 0:16, :],
                                 prod2[:, 16:32, :])
            ptrf = tailp.tile([J, 8, 2 * H], F32)
            nc.vector.tensor_add(ptrf, qq[:, 0:8, :], qq[:, 8:16, :])
            wt = 8
            while wt > 1:
                wt //= 2
                nc.vector.tensor_add(ptrf[:, 0:wt, :], ptrf[:, 0:wt, :],
                                     ptrf[:, wt : 2 * wt, :])
            pooled = ptrf[:, 0:1, :].rearrange("j one p -> j (one p)")
            if "DBG_LOG" in outs:
                nc.sync.dma_start(outs["DBG_LOG"], logits)
                nc.sync.dma_start(outs["DBG_PO"], pooled)

            # context norm across each sample's (d, 2h) block
            pooled2 = tailp.tile([J, 2 * H], F32)
            nc.scalar.activation(out=pooled2, in_=pooled, func=AF.Square)
            sel = tailp.tile([J, BLOC], F32)
            nc.sync.dma_start(sel, ins["SEL"])
            pstat = tailps.tile([BLOC, 2 * G4], F32, tag="stats")
            nc.tensor.matmul(pstat[:, 0 : 2 * H], lhsT=sel, rhs=pooled,
                             start=True, stop=False)
            nc.tensor.matmul(pstat[:, 2 * H : 4 * H], lhsT=sel, rhs=pooled2,
                             start=False, stop=True)
            s1 = tailp.tile([BLOC, 1], F32)
            nc.vector.tensor_reduce(s1, pstat[:, 0 : 2 * H],
                                    axis=mybir.AxisListType.X, op=ALU.add)
            s2 = tailp.tile([BLOC, 1], F32)
            nc.vector.tensor_reduce(s2, pstat[:, 2 * H : 4 * H],
                                    axis=mybir.AxisListType.X, op=ALU.add)
            stats2 = tailp.tile([BLOC, 2], F32)
            nc.scalar.mul(stats2[:, 0:1], s1, 1.0 / NORM_N)      # mean
            q = tailp.tile([BLOC, 1], F32)
            nc.vector.tensor_mul(q, s1, stats2[:, 0:1])          # sum*mean
            v = tailp.tile([BLOC, 1], F32)
            nc.vector.tensor_tensor(v, s2, q, op=ALU.subtract)
            sd = tailp.tile([BLOC, 1], F32)
            nc.scalar.activation(out=sd, in_=v, func=AF.Sqrt,
                                 scale=1.0 / (NORM_N - 1))
            nc.vector.reciprocal(stats2[:, 1:2], sd)             # rstd
            selt = tailp.tile([BLOC, J], F32)
            nc.sync.dma_start(selt, ins["SELT"])
            pmb = tailps.tile([J, 2], F32, tag="mb")
            nc.tensor.matmul(pmb, lhsT=selt, rhs=stats2, start=True, stop=True)
            mb = tailp.tile([J, 2], F32)
            nc.vector.tensor_copy(mb, pmb)
            dwt = tailp.tile([J, 2 * H], F32)
            nc.sync.dma_start(dwt[0:D, :], DW)
            nc.sync.dma_start(dwt[D:J, :], DW)
            dbt = tailp.tile([J, 2 * H], F32)
            nc.sync.dma_start(dbt[0:D, :], DB)
            nc.sync.dma_start(dbt[D:J, :], DB)
            t1 = tailp.tile([J, 2 * H], F32)
            nc.vector.tensor_scalar(t1, pooled, mb[:, 0:1], mb[:, 1:2],
                                    op0=ALU.subtract, op1=ALU.mult)
            t2 = tailp.tile([J, 2 * H], F32)
            nc.vector.tensor_mul(t2, t1, dwt)
            t3 = tailp.tile([J, 2 * H], F32)
            nc.vector.tensor_add(t3, t2, dbt)
            nc.sync.dma_start(OUT, t3)


def build_program():
    nc = bacc.Bacc("TRN2", target_bir_lowering=False, debug=False)
    ins = {
        "XT": nc.dram_tensor("XT", [NF, R], F32, kind="ExternalInput").ap(),
        "WTT": nc.dram_tensor("WTT", [NF, TS], F32, kind="ExternalInput").ap(),
        "BT": nc.dram_tensor("BT", [TS, 1], F32, kind="ExternalInput").ap(),
        "WIH": nc.dram_tensor("WIH", [TS + 1, 2, G4], BF16, kind="ExternalInput").ap(),
        "WHH": nc.dram_tensor("WHH", [H, 2, G4], BF16, kind="ExternalInput").ap(),
        "ONES": nc.dram_tensor("ONES", [1, R], BF16, kind="ExternalInput").ap(),
        "DW": nc.dram_tensor("DW", [D, 2 * H], F32, kind="ExternalInput").ap(),
        "SEL": nc.dram_tensor("SEL", [J, BLOC], F32, kind="ExternalInput").ap(),
        "SELT": nc.dram_tensor("SELT", [BLOC, J], F32, kind="ExternalInput").ap(),
        "DB": nc.dram_tensor("DB", [D, 2 * H], F32, kind="ExternalInput").ap(),
    }
    outs = {
        "OUT": nc.dram_tensor("OUT", [J, 2 * H], F32, kind="ExternalOutput").ap(),
    }
    with tile.TileContext(nc) as tc:
        emit(tc, ins, outs)
    nc.compile()
    return nc


def _prep_dir(Wih, Whh, bih, bhh):
    # gate order (i,f,o,g); the g block is pre-scaled by 2 so the kernel can
    # evaluate tanh(g) as 2*sigmoid(2g)-1 inside the fused sigmoid op
    wihT = Wih.T.reshape(TS, 4, H)[:, PERM, :].reshape(TS, G4).copy()
    biasr = (bih + bhh).reshape(4, H)[PERM, :].reshape(G4).copy()
    wihT[:, 3 * H :] *= 2.0
    biasr[3 * H :] *= 2.0
    wih65 = np.concatenate([wihT, biasr[None, :]], axis=0).astype(BF16NP)
    whhT = Whh.T.reshape(H, 4, H)[:, PERM, :].reshape(H, G4).copy()
    whhT[:, 3 * H :] *= 2.0
    whhT = whhT.astype(BF16NP)
    return wih65, whhT


def prep_inputs(X, W_t, b_t, Wih_f, Whh_f, bih_f, bhh_f,
                Wih_b, Whh_b, bih_b, bhh_b, diag_w, diag_b):
    wih_f, whh_f = _prep_dir(Wih_f, Whh_f, bih_f, bhh_f)
    wih_b, whh_b = _prep_dir(Wih_b, Whh_b, bih_b, bhh_b)
    shared = {
        "WTT": np.ascontiguousarray(W_t.T, dtype=np.float32),
        "BT": np.ascontiguousarray(b_t.reshape(TS, 1), dtype=np.float32),
        "WIH": np.ascontiguousarray(np.stack([wih_f, wih_b], axis=1)),
        "WHH": np.ascontiguousarray(np.stack([whh_f, whh_b], axis=1)),
        "ONES": np.ones((1, R), dtype=BF16NP),
        "SEL": np.kron(np.eye(BLOC, dtype=np.float32), np.ones((D, 1), np.float32)),
        "SELT": np.kron(np.eye(BLOC, dtype=np.float32), np.ones((1, D), np.float32)),
        "DW": np.ascontiguousarray(diag_w.reshape(D, 2 * H), dtype=np.float32),
        "DB": np.ascontiguousarray(diag_b.reshape(D, 2 * H), dtype=np.float32),
    }
    in_maps = []
    for i in range(NCORES):
        xt = np.ascontiguousarray(
            X[i * BLOC : (i + 1) * BLOC].transpose(3, 1, 0, 2).reshape(NF, R),
            dtype=np.float32,
        )
        m = {"XT": xt}
        m.update(shared)
        in_maps.append(m)
    return in_maps


def kernel(**inputs):
    inputs = {k: np.asarray(v, dtype=np.float32) for k, v in inputs.items()}
    in_maps = prep_inputs(**inputs)
    nc = build_program()
    res = run_bass_kernel_spmd(nc, in_maps, list(range(NCORES)))
    out = np.concatenate(
        [res.results[i]["OUT"].reshape(BLOC, D, 2 * H) for i in range(NCORES)],
        axis=0,
    )
    return np.ascontiguousarray(out, dtype=np.float32)


if __name__ == "__main__":
    nc = build_program()
    print("program built ok")
